# revision 24
# baseline (speedup 1.0000x reference)
"""Trainium2 Bass kernel for nn_Matposer_51007031608225.

Key algebraic insight: the reference computes fmap = einsum('bld,ble->bde')
(a [512,300,300] bmm) but keeps only diagonal(fmap, axis1=0, axis2=1), i.e.
fmap[k,k,:] for k < 300.  So per batch-index k only

    diagT[k, e] = sum_l a_k[l] * (scale*emb1[x1[k,l],e] + pe[l,e])
    a_k[l]      = scale*emb2[x2[k,l],k] + pe[l,k]

is needed - a [300x512]@[512] matvec per k instead of the full bmm.  The
dominant cost is gathering 300*512 embedding rows, data-parallel over k
across the 8 cores (38 k's per core).

This version (vs the f32 baseline, 126us -> 96us under the TimelineSim
cost model):
  - emb1 rows are gathered in FP16 (384-col padded rows = 768B descriptors
    instead of 1280B f32): the dominant DMA stream drops ~40%.  FP16 (not
    bf16!) because the head contracts 300 diag entries of magnitude up to
    ~262; bf16's 8-bit mantissa propagates to ~25% softmax error while
    fp16's 11 bits give ~1e-2.
  - phase-1 matmuls run in fp16 (1 PE-cycle/row instead of 4 for f32), in
    FLIPPED form: e on PSUM partitions, k on the free dim, so each per-k
    matvec is 12 N=1 matmuls (nearly free in the cost model, and the PSUM
    base partition is always 0).  The pe/D-term matmuls accumulate per
    chunk into a second PSUM tile (pe4b is pre-divided by SCALE so s_ab
    serves as the rhs); emb2 gathers issue BEFORE emb1 per chunk so the
    a-value chain hides under the emb1 transfer.
  - the merged diag piece is PE-transposed to k-major f32 [38, 384] so
    phase 2 needs no transposes: its mm1 contracts the 8 per-core pieces
    as 8 independent 38-row chunks.
  - phase 2 (the tiny [300,300] MLP head + softmax, e-sharded 38 cols/core)
    folds b2 into mm2 via a constant-1 h-row (b1[300]=1 drives it through
    the relu); max-subtraction is kept (logits reach +-200).

Precision: fp16 x fp16 products with f32 PSUM accumulation and an all-f32
head; measured rel-err ~1.0e-2 against the 2e-2 gate.
"""

import numpy as np
from contextlib import ExitStack

import concourse.bass as bass
import concourse.bacc as bacc
import concourse.tile as tile
import concourse.mybir as mybir
from concourse.bass_utils import run_bass_kernel_spmd
from concourse.masks import make_identity

F32 = mybir.dt.float32
F16 = mybir.dt.float16
I16 = mybir.dt.int16

D = 300          # d_model
L = 512          # sequence length
V = 32000        # vocab
OUT = 4
NCORES = 8
NK = 38          # k's per core (8*38 = 304 >= 300)
EP = 384         # padded emb1 row (bf16) -> 768B descriptors (mult of 256B)
E2P = 64         # padded per-core emb2 channel slab (f32) -> 256B
CHUNK_SIZES = [4] * 9 + [2]     # k's per gather chunk; sums to NK
SCALE = float(np.sqrt(np.float32(D)))
H16 = np.float16


# ---------------------------------------------------------------- phase 1

def _build_phase1():
    # three SWDGE queues: emb1 row-gathers alternate q0/q1, emb2 slab-gathers
    # on q2 so the small desc-bound emb2 stream rides under the byte-bound
    # emb1 stream
    nc = bacc.Bacc("TRN2", target_bir_lowering=False, debug=False,
                   num_devices=NCORES, num_swdge_queues=3)

    emb1b = nc.dram_tensor("emb1b", [V, EP], F16, kind="ExternalInput").ap()
    emb2sl = nc.dram_tensor("emb2sl", [V, E2P], F32, kind="ExternalInput").ap()
    x1w_d = nc.dram_tensor("x1w", [128, NK * 32], I16, kind="ExternalInput").ap()
    x2w_d = nc.dram_tensor("x2w", [128, NK * 32], I16, kind="ExternalInput").ap()
    pe4b_d = nc.dram_tensor("pe4b", [128, 4 * D], F16, kind="ExternalInput").ap()
    pec_d = nc.dram_tensor("pec", [128, NK * 4], F32, kind="ExternalInput").ap()
    diagK_d = nc.dram_tensor("diagK", [NK, 384], F32, kind="ExternalOutput").ap()

    EC3 = [(0, 128), (128, 128), (256, 44)]   # e-chunks

    with tile.TileContext(nc) as tc, ExitStack() as ctx:
        cpool = ctx.enter_context(tc.tile_pool(name="consts", bufs=1))
        g1pool = ctx.enter_context(tc.tile_pool(name="g1", bufs=4))
        g2pool = ctx.enter_context(tc.tile_pool(name="g2", bufs=4))
        spool = ctx.enter_context(tc.tile_pool(name="small", bufs=1))
        psp = ctx.enter_context(tc.tile_pool(name="ps", bufs=1, space="PSUM"))

        x2w = cpool.tile([128, NK * 32], I16)
        nc.sync.dma_start(x2w[:, :128], x2w_d[:, :128])
        nc.sync.dma_start(x2w[:, 128:], x2w_d[:, 128:])
        x1w = cpool.tile([128, NK * 32], I16)
        nc.sync.dma_start(x1w[:], x1w_d[:])
        pec = cpool.tile([128, NK * 4], F32)
        nc.sync.dma_start(pec[:], pec_d[:])
        pe4b = cpool.tile([128, 4 * D], F16)
        nc.scalar.dma_start(pe4b[:], pe4b_d[:])
        idt = cpool.tile([128, 128], F32)
        make_identity(nc, idt[:])

        a_raw = spool.tile([128, NK * 4], F32)
        a_full = spool.tile([128, NK * 4], F32)
        s_ab = spool.tile([128, NK * 4], F16)

        # e-major accumulators: pkG[em][e', k] = sum_l s_a[k,l]*emb1[x1,e]
        pkG = [psp.tile([128, NK], F32, name=f"pkG{m}", tag=f"pg{m}") for m in range(3)]
        pkPE = [psp.tile([128, NK], F32, name=f"pkPE{m}", tag=f"pp{m}") for m in range(3)]
        pT = psp.tile([NK, 128], F32, name="pT0", tag="pt")

        so = spool.tile([128, 3 * NK], F32)
        pesb = spool.tile([128, 3 * NK], F32)
        outk = spool.tile([NK, 384], F32)

        off = 0
        for ci, ch in enumerate(CHUNK_SIZES):
            ni = ch * L
            g2 = g2pool.tile([128, ch * 4 * E2P], F32, tag="g2")
            nc.gpsimd.dma_gather(
                out_ap=g2[:].rearrange("p (c e) -> p c e", e=E2P),
                in_ap=emb2sl[:],
                idxs_ap=x2w[:, off * 32:(off + ch) * 32],
                num_idxs=ni,
                num_idxs_reg=ni,
                elem_size=E2P,
                single_packet=False,
                queue_num=2,
            )
            g1 = g1pool.tile([128, ch * 4 * EP], F16, tag="g1")
            nc.gpsimd.dma_gather(
                out_ap=g1[:].rearrange("p (c e) -> p c e", e=EP),
                in_ap=emb1b[:],
                idxs_ap=x1w[:, off * 32:(off + ch) * 32],
                num_idxs=ni,
                num_idxs_reg=ni,
                elem_size=EP,
                single_packet=False,
                queue_num=ci % 2,
            )
            g2v = g2[:].rearrange("p (c e) -> p c e", e=E2P)
            for kk in range(ch):
                klc = off + kk   # core-local k == channel in emb2sl
                nc.vector.tensor_copy(
                    a_raw[:, klc * 4:(klc + 1) * 4],
                    g2v[:, kk * 4:(kk + 1) * 4, klc],
                )
            # a_full = scale*a_raw + pe_cols ; s_ab = bf16(scale*a_full)
            cols = slice(off * 4, (off + ch) * 4)
            nc.vector.tensor_scalar_mul(a_full[:, cols], a_raw[:, cols], SCALE)
            nc.vector.tensor_tensor(
                out=a_full[:, cols], in0=a_full[:, cols], in1=pec[:, cols],
                op=mybir.AluOpType.add,
            )
            nc.vector.tensor_scalar_mul(s_ab[:, cols], a_full[:, cols], SCALE)

            # flipped matvec: pkG[em][0:en, klc] += g1rows[:, e-chunk]^T @ s_a-col
            # (N=1 matmuls: nearly free on the PE, base partition always 0)
            for kk in range(ch):
                klc = off + kk
                for c in range(4):
                    r0 = (kk * 4 + c) * EP
                    for em, (e0, en) in enumerate(EC3):
                        nc.tensor.matmul(
                            out=pkG[em][0:en, klc:klc + 1],
                            lhsT=g1[:, r0 + e0: r0 + e0 + en],
                            rhs=s_ab[:, klc * 4 + c: klc * 4 + c + 1],
                            start=(c == 0),
                            stop=(c == 3),
                        )
            # in-loop pe-term: pkPE[em][:, chunk cols] = sum_l a*pe[l,e]
            # (pe4b is pre-divided by SCALE on the host so s_ab works as rhs)
            sav = s_ab[:].rearrange("p (k c) -> p c k", c=4)
            for em, (e0, en) in enumerate(EC3):
                for c in range(4):
                    nc.tensor.matmul(
                        out=pkPE[em][0:en, off:off + ch],
                        lhsT=pe4b[:, c * D + e0: c * D + e0 + en],
                        rhs=sav[:, c, off:off + ch],
                        start=(c == 0),
                        stop=(c == 3),
                    )
            off += ch

        # merge G+PE, transpose each e-chunk to k-major, emit [NK, 384] bf16
        for em, (e0, en) in enumerate(EC3):
            ceng = [nc.scalar, nc.vector, nc.scalar][em]
            if em == 1:
                ceng.tensor_copy(pesb[0:en, em * NK:(em + 1) * NK],
                                 pkPE[em][0:en, :])
            else:
                ceng.copy(pesb[0:en, em * NK:(em + 1) * NK], pkPE[em][0:en, :])
            nc.vector.tensor_tensor(out=so[0:en, em * NK:(em + 1) * NK],
                               in0=pkG[em][0:en, :],
                               in1=pesb[0:en, em * NK:(em + 1) * NK],
                               op=mybir.AluOpType.add)
            nc.tensor.transpose(pT[0:NK, 0:en],
                                so[0:en, em * NK:(em + 1) * NK],
                                idt[0:en, 0:en])
            oeng = [nc.vector, nc.scalar, nc.vector][em]
            if em == 1:
                oeng.copy(outk[:, em * 128: em * 128 + en], pT[0:NK, 0:en])
            else:
                oeng.tensor_copy(outk[:, em * 128: em * 128 + en],
                                 pT[0:NK, 0:en])
            nc.sync.dma_start(diagK_d[:, em * 128: em * 128 + en],
                              outk[:, em * 128: em * 128 + en])

    nc.compile()
    return nc


# ---------------------------------------------------------------- phase 2

EC = 38   # e-columns of the head computed per core (8*38 = 304 >= 300)
NKP = 384   # padded j rows (3*128) for the w2/b1 chunked loads


def _build_phase2():
    """e-sharded head: every core gets the full diag rows (as 8 k-major
    38-row pieces) but only its own 38-column e-slice; computes [38, 4]
    output rows.  The k/j contraction runs piece-wise (8 x 38 rows for mm1,
    3 x 128 chunks for mm2); padded rows are zero on the host side."""
    nc = bacc.Bacc("TRN2", target_bir_lowering=False, debug=False,
                   num_devices=NCORES)

    # dw = [dS | w1p]: dS[r, c*EC + e'] = diag[k = 38c + r, e0 + e'];
    # w1p[r, c*304 + j] = w1[j, 38c + r] (zero for k >= 300) - one DMA
    dw_d = nc.dram_tensor("dw", [NK, NCORES * EC + NCORES * 304],
                          mybir.dt.float32r, kind="ExternalInput").ap()
    # w2b[j, o] chunks: [304, 4] bf16 (zero for j >= 300)
    w2b_d = nc.dram_tensor("w2b", [NKP, OUT], F32, kind="ExternalInput").ap()
    b1_d = nc.dram_tensor("b1p", [NKP, 1], F32, kind="ExternalInput").ap()
    out_d = nc.dram_tensor("out", [EC, OUT], F32, kind="ExternalOutput").ap()

    JC = [(0, 128), (128, 128), (256, 45)]   # j=300 is the ones-row (b1=1)

    with tile.TileContext(nc) as tc, ExitStack() as ctx:
        pool = ctx.enter_context(tc.tile_pool(name="p2", bufs=1))
        psum = ctx.enter_context(tc.tile_pool(name="ps2", bufs=1, space="PSUM"))

        dw = pool.tile([NK, NCORES * EC + NCORES * 304], mybir.dt.float32r)
        nc.sync.dma_start(dw[:], dw_d[:])
        DSB = NCORES * EC          # w1p columns start here

        w2b = pool.tile([128, 3 * OUT], F32)
        nc.scalar.dma_start(w2b[:].rearrange("p (c o) -> p c o", o=OUT),
                            w2b_d[:].rearrange("(c p) o -> p c o", p=128))
        b1t = pool.tile([128, 3], F32)
        nc.scalar.dma_start(b1t[:].rearrange("p (c x) -> p c x", x=1),
                            b1_d[:].rearrange("(c p) x -> p c x", p=128))

        # hT[j, e'] = relu(sum_k w1[j,k] diag[k, e0+e'] + b1[j])
        hT = []
        for jm, (j0, jn) in enumerate(JC):
            ph = psum.tile([128, EC], F32, tag=f"ph{jm}", space="PSUM")
            for c in range(NCORES):
                nc.tensor.matmul(
                    out=ph[:jn, :],
                    lhsT=dw[:, DSB + c * 304 + j0: DSB + c * 304 + j0 + jn],
                    rhs=dw[:, c * EC:(c + 1) * EC],
                    start=(c == 0), stop=(c == NCORES - 1))
            th = pool.tile([128, EC], F32, tag=f"h{jm}")
            nc.scalar.activation(th[:jn, :], ph[:jn, :],
                                 mybir.ActivationFunctionType.Relu,
                                 bias=b1t[:jn, jm:jm + 1], scale=1.0)
            hT.append(th)


        # logits[e', o] = sum_j hT[j, e'] w2[j, o] + b2[o]
        pl = psum.tile([128, OUT], F32, tag="pl", space="PSUM")
        JC2 = [(0, 128), (128, 128), (256, 45)]   # row 44 of chunk 2 = ones
        for jm, (j0, jn) in enumerate(JC2):
            nc.tensor.matmul(
                out=pl[:EC, :],
                lhsT=hT[jm][:jn, :],
                rhs=w2b[:jn, jm * OUT:(jm + 1) * OUT],
                start=(jm == 0), stop=(jm == 2))
        nmax = pool.tile([128, 1], F32, tag="nm")
        nc.vector.reduce_max(nmax[:EC, :], pl[:EC, :],
                             axis=mybir.AxisListType.X, negate=True)
        ex = pool.tile([128, OUT], F32, tag="ex")
        ssum = pool.tile([128, 1], F32, tag="ss")
        nc.scalar.activation(ex[:EC, :], pl[:EC, :],
                             mybir.ActivationFunctionType.Exp,
                             bias=nmax[:EC, :], scale=1.0,
                             accum_out=ssum[:EC, :])
        rcp = pool.tile([128, 1], F32, tag="rc")
        nc.vector.reciprocal(rcp[:EC, :], ssum[:EC, :])
        sm = pool.tile([128, OUT], F32, tag="so")
        nc.vector.tensor_scalar_mul(sm[:EC, :], ex[:EC, :], rcp[:EC, :])
        nc.sync.dma_start(out_d[:], sm[:EC, :])

    nc.compile()
    return nc


_CACHE = {}


def _phase1():
    if "p1" not in _CACHE:
        _CACHE["p1"] = _build_phase1()
    return _CACHE["p1"]


def _phase2():
    if "p2" not in _CACHE:
        _CACHE["p2"] = _build_phase2()
    return _CACHE["p2"]


# ---------------------------------------------------------------- host glue

def _pe_table():
    pos = np.arange(L, dtype=np.float32)[:, None]
    div = np.exp(np.arange(0, D, 2, dtype=np.float32)
                 * np.float32(-np.log(10000.0) / D))
    pe = np.zeros((L, D), dtype=np.float32)
    pe[:, 0::2] = np.sin(pos * div)
    pe[:, 1::2] = np.cos(pos * div)
    return pe


def _wrap_idx(rows):
    """rows [nk, 512] -> int16 [128, nk*32] in dma_gather's wrapped layout
    (per CHUNK_SIZES blocks; idx i of a chunk sits at [i%16, blockcol+i//16],
    replicated down all 128 partitions)."""
    out = np.zeros((16, rows.shape[0] * 32), dtype=np.int16)
    off = 0
    for ch in CHUNK_SIZES:
        seq = rows[off:off + ch].reshape(-1)            # ch*512
        out[:, off * 32:(off + ch) * 32] = seq.reshape(-1, 16).T
        off += ch
    return np.tile(out, (8, 1))


def kernel(x1, x2, emb1, emb2, w1, b1, w2, b2, _trace=(False, False)):
    x1 = np.asarray(x1); x2 = np.asarray(x2)
    emb1 = np.asarray(emb1, dtype=np.float32)
    emb2 = np.ascontiguousarray(np.asarray(emb2, dtype=np.float32))
    w1 = np.asarray(w1, dtype=np.float32); b1 = np.asarray(b1, dtype=np.float32)
    w2 = np.asarray(w2, dtype=np.float32); b2 = np.asarray(b2, dtype=np.float32)

    pe = _pe_table()
    emb1b = np.zeros((V, EP), dtype=H16)
    emb1b[:, :D] = emb1.astype(H16)

    # pe4b: [p, c*300+e] = pe[c*128+p, e]  (bf16)
    pe4b = np.ascontiguousarray(
        pe.reshape(4, 128, D).transpose(1, 0, 2).reshape(128, 4 * D) / SCALE).astype(H16)

    in_maps = []
    for core in range(NCORES):
        k0 = NK * core
        kidx = np.arange(k0, k0 + NK)
        x1w = _wrap_idx(x1[k0:k0 + NK].astype(np.int64))
        x2w = _wrap_idx(x2[k0:k0 + NK].astype(np.int64))
        nch = min(NK, max(0, D - k0))        # real channels for this core
        emb2sl = np.zeros((V, E2P), dtype=np.float32)
        emb2sl[:, :nch] = emb2[:, k0:k0 + nch]
        # pe_cols[p, kk*4+c] = pe[c*128+p, k0+kk] (0 when k >= 300)
        pec = np.zeros((128, NK * 4), dtype=np.float32)
        valid = kidx < D
        pev = pe[:, kidx[valid]].reshape(4, 128, valid.sum())  # [c, p, kk]
        pec_v = pec.reshape(128, NK, 4)
        pec_v[:, valid, :] = pev.transpose(1, 2, 0)
        in_maps.append({
            "emb1b": emb1b,
            "emb2sl": emb2sl,
            "x1w": x1w,
            "x2w": x2w,
            "pe4b": pe4b,
            "pec": pec,
        })

    res1 = run_bass_kernel_spmd(_phase1(), in_maps,
                                core_ids=list(range(NCORES)), trace=_trace[0])
    # diag pieces: piece c = [38 k-rows, 300 e-cols] (bf16)
    pieces = [np.asarray(r["diagK"]) for r in res1.results]

    # phase-2 host marshaling (pure layout): w1 pieces, w2 chunks, biases
    w1T = w1.T  # [k, j]
    w1p = np.zeros((NK, NCORES * 304), dtype=np.float32)
    for c in range(NCORES):
        k0 = c * NK
        kn = min(NK, max(0, D - k0))
        if kn > 0:
            w1p[:kn, c * 304:c * 304 + D] = w1T[k0:k0 + kn, :]
    w2b = np.zeros((NKP, OUT), dtype=np.float32)
    w2b[:D] = w2.T
    w2b[D] = b2                      # ones-row bias trick (j = 300)
    b1p = np.zeros((NKP, 1), dtype=np.float32)
    b1p[:D, 0] = b1
    b1p[D, 0] = 1.0                  # ones-row for the b2 fold

    in2_maps = []
    for core in range(NCORES):
        e0 = EC * core
        ne = min(EC, max(0, D - e0))
        dS = np.zeros((NK, NCORES * EC), dtype=np.float32)
        for c in range(NCORES):
            dS[:, c * EC:c * EC + ne] = pieces[c][:, e0:e0 + ne]
        in2_maps.append({
            "dw": np.concatenate([dS, w1p], axis=1),
            "w2b": w2b,
            "b1p": b1p,
        })
    res2 = run_bass_kernel_spmd(_phase2(), in2_maps,
                                core_ids=list(range(NCORES)), trace=_trace[1])
    out = np.concatenate([np.asarray(r["out"]) for r in res2.results])[:D]
    out = np.ascontiguousarray(out.astype(np.float32))

    if _trace[0] or _trace[1]:
        kernel._last_exec_ns = (res1.exec_time_ns, res2.exec_time_ns)
        kernel._last_results = (res1, res2)
    return out
